# revision 18
# baseline (speedup 1.0000x reference)
"""ColBERT intra-batch MaxSim scoring kernel for 8 Trainium2 NeuronCores.

Math (see reference):
  Q = l2norm(q_hidden @ W.T)                       [B, LQ, DIM]
  D = l2norm(d_hidden @ W.T); D masked             [B, LD, DIM]
  sim[b,c,q,k] = Q[b,q]·D[c,k]; masked k -> -inf
  out[b,c] = sum_q max_k sim

Sharding: docs (dim c) are sharded 16-per-core; q_hidden/W replicated.
Each core computes its [B, 16] slice of the score matrix.

Design notes (v2):
  * The DVE reduce_max over the sim matrix is the hard floor: every sim
    element must pass through the vector engine once at ~1 elem/cycle/lane
    (~81us/core for 4096 q-tokens x 2432 doc-token cols / 128 lanes); no
    other engine can reduce along the free axis from PSUM. The kernel is
    therefore organized so DVE runs reduces back-to-back and everything
    else hides under them.
  * ALL activations/weights move and multiply in bf16 (fp32 PE matmuls are
    ~3.5x slower and fp32 DMA is 2x the bytes); PSUM accumulation stays
    fp32. Host casts inputs to bf16 after transposing.
  * Host pre-transposes activations to [HID, tokens] so every matmul has
    its contraction dim on partitions. The doc mask is folded away on the
    host: valid tokens gathered front, tail padded with dup of the first
    valid token (dups never change a max) -> no masking on device.
  * Q is NOT normalized before the sim matmul: max_k is invariant under a
    positive per-query scale, so 1/|Q| is folded into the block-ones
    lhsT of the final query-sum matmul.
  * D IS normalized before the sim matmul (1/|d_k| does not commute with
    max_k): ones-matmul sumsq -> sqrt (ACT) + fast reciprocal (DVE) ->
    K=1 ones outer-product matmul broadcasts 1/|D| to 128 partitions ->
    DVE multiply straight out of the projection PSUM into bf16 SBUF.
  * DVE reads at most ONE PSUM operand per instruction (HW rule), and
    only ~1 elem/cycle in every mode, so no fold tricks help; the plain
    grouped reduce (one per sim half-tile) is optimal.
"""

import os

import numpy as np

B, LQ, LD, HID, DIM = 128, 32, 256, 768, 128
NCORES = 8
DPC = B // NCORES          # docs per core
TQ = B * LQ                # total query tokens
KC = HID // 128            # contraction chunks for the projection


def _chunks(total, step):
    """[(off, len)] cut at `step` boundaries — a matmul's PSUM output must
    stay inside a single 512-float bank, so chunks may never straddle one."""
    return [(o, min(step, total - o)) for o in range(0, total, step)]


def _build_program(NV_A, NV_B):
    import concourse.bass as bass  # noqa: F401
    import concourse.tile as tile
    from concourse import bacc, mybir

    f32 = mybir.dt.float32
    bf16 = mybir.dt.bfloat16
    AF = mybir.ActivationFunctionType
    AX = mybir.AxisListType
    ALU = mybir.AluOpType

    # two doc classes: the 8 longest docs (padded to NV_A) in half A, the 8
    # shortest (padded to NV_B <= NV_A) in half B — same instruction count,
    # ~8% fewer sim/reduce elements than uniform padding
    NVH = [(DPC // 2) * NV_A, (DPC // 2) * NV_B]
    HB = [0, NVH[0]]               # half base offsets
    NVS = [NV_A, NV_B]
    NVT = NVH[0] + NVH[1]          # compacted doc tokens per core
    NQCH = TQ // 512        # q-projection column chunks
    NTT = TQ // 128         # sim lhsT tiles (query-token tiles)
    BPT = 128 // LQ         # batch entries per query-token tile
    QG = 1024               # qt DMA column-group width

    nc = bacc.Bacc(
        "TRN2",
        target_bir_lowering=False,
        debug=False,
        num_devices=NCORES,
    )

    qT_d = nc.dram_tensor("qT", [HID, TQ], bf16, kind="ExternalInput")
    dT_d = nc.dram_tensor("dT", [HID, NVT], bf16, kind="ExternalInput")
    wT_d = nc.dram_tensor("wT", [128, KC, DIM], bf16, kind="ExternalInput")
    qso_d = nc.dram_tensor("qso", [128, BPT], f32, kind="ExternalInput")
    onescol_d = nc.dram_tensor("onescol", [128, 1], bf16, kind="ExternalInput")
    onesrow_d = nc.dram_tensor("onesrow", [1, 128], bf16, kind="ExternalInput")
    out_d = nc.dram_tensor("out", [B, DPC], f32, kind="ExternalOutput")

    with tile.TileContext(nc) as tc, tc.tile_pool(name="persist", bufs=1) as per:
        # --- constants + persistent SBUF tensors ---------------------------
        wt = per.tile([128, KC, DIM], bf16, name="wt")
        qso = per.tile([128, BPT], f32, name="qso")
        onescol = per.tile([128, 1], bf16, name="onescol")
        onesrow = per.tile([1, 128], bf16, name="onesrow")
        QT = per.tile([128, TQ], bf16, name="QT")         # q-proj [d, t] unnormalized
        DTn = per.tile([128, NVT], bf16, name="DTn")      # normalized d-proj
        invnQ = per.tile([128, NTT], f32, name="invnQ")   # 1/|Q| per query token
        normQ = per.tile([128, NTT], f32, name="normQ")
        lhsQ = per.tile([128, NTT, BPT], f32, name="lhsQ")  # blockones * 1/|Q|
        ssqD_row = per.tile([1, NVT], f32, name="ssqD_row")
        invnD32 = per.tile([1, NVT], f32, name="invnD32")
        invnD_row = per.tile([1, NVT], bf16, name="invnD_row")
        rowtmp = per.tile([1, NVT], f32, name="rowtmp")
        DrawB = per.tile([128, NVH[1]], bf16, name="DrawB")  # raw B proj
        outstage = per.tile([BPT, NTT * DPC], f32, name="outstage")

        # constants go first on the gpsimd queue so wt is resident before
        # the first projection matmul; dT halves ride two queues in parallel
        # (sync: half A, gpsimd: half B) so phase D is DMA-gated for only
        # ~2MB per queue; qt jg0 rides the otherwise-idle scalar queue
        nc.gpsimd.dma_start(wt[:], wT_d[:, :, :])
        nc.gpsimd.dma_start(qso[:], qso_d[:, :])
        nc.gpsimd.dma_start(onescol[:], onescol_d[:, :])
        nc.gpsimd.dma_start(onesrow[:], onesrow_d[:, :])

        # ---------------- phase D: project + normalize doc tokens ---------
        # dT halves land on the sync queue (nothing else competes there);
        # qt column groups land on the gpsimd queue. Scalar/vector issue no
        # DMAs — their cycles belong to copies and reduces.
        qs_stack = tc.tile_pool(name="qt_pool", bufs=1)
        qt_pool = qs_stack.__enter__()
        qts = {}

        def load_jg(jg, engs=None):
            engs = engs or [nc.gpsimd]
            for k in range(KC):
                t_ = qt_pool.tile(
                    [128, QG], bf16, name=f"qt{k}_{jg}", tag=f"qt{k}",
                    bufs=2,
                )
                engs[k % len(engs)].dma_start(
                    t_[:], qT_d[k * 128:(k + 1) * 128, jg * QG:(jg + 1) * QG]
                )
                qts[(k, jg)] = t_

        # NOTE: scalar may only issue a FEW upfront DMAs — a back-pressured
        # issue blocks its in-order queue and stalls the phase-D squares
        # (measured +25us on the ramp with 12 queued issues).

        with (
            tc.tile_pool(name="dt_pool", bufs=1) as dt_pool,
            tc.tile_pool(name="psD", bufs=1, space="PSUM") as psD,
            tc.tile_pool(name="ssD", bufs=1, space="PSUM") as ssD,
            tc.tile_pool(name="sqD_pool", bufs=2) as sqD_pool,
            tc.tile_pool(name="psB", bufs=1, space="PSUM") as psB,
        ):
            # Per-queue DMA rate is only ~60-90GB/s (small strided rows),
            # so only ramp-critical bytes go upfront, spread over all three
            # queues: dT half A + qt group 0, then dT half B + qt group 1;
            # later qt groups trickle mid-loop on gpsimd. Scalar's issue
            # count stays small so its in-order ACT queue isn't blocked.
            dts = {}
            for h in range(2):
                for k in range(KC):
                    dts[(k, h)] = dt_pool.tile(
                        [128, NVH[h]], bf16, name=f"dt{k}_{h}", tag=f"dt{k}_{h}"
                    )

            def load_dt(h, ks, eng):
                for k in ks:
                    eng.dma_start(
                        dts[(k, h)][:],
                        dT_d[k * 128:(k + 1) * 128, HB[h]:HB[h] + NVH[h]],
                    )

            def load_jg_ks(jg, ks, eng):
                for k in ks:
                    t_ = qt_pool.tile(
                        [128, QG], bf16, name=f"qt{k}_{jg}", tag=f"qt{k}",
                        bufs=2,
                    )
                    eng.dma_start(
                        t_[:], qT_d[k * 128:(k + 1) * 128,
                                    jg * QG:(jg + 1) * QG]
                    )
                    qts[(k, jg)] = t_

            load_jg_ks(0, [0, 1, 2], nc.scalar)
            load_dt(0, [0, 2], nc.sync)
            load_dt(0, [1, 3, 5], nc.gpsimd)
            load_dt(0, [4], nc.scalar)
            load_jg_ks(0, [3, 4, 5], nc.sync)
            load_dt(1, [5], nc.scalar)
            load_dt(1, [1, 3], nc.gpsimd)
            load_dt(1, [0, 2, 4], nc.sync)
            load_jg_ks(1, [0, 2, 4], nc.gpsimd)
            load_jg_ks(1, [1, 3, 5], nc.sync)

            for h in range(2):
                base = HB[h]
                h_chunks = _chunks(NVH[h], 512)
                psd = psD.tile([128, NVH[h]], f32, name="psd", tag=f"psd{h}")
                for k in range(KC):
                    for (off, ln) in h_chunks:
                        nc.tensor.matmul(
                            psd[:, off:off + ln],
                            wt[:, k, :],
                            dts[(k, h)][:, off:off + ln],
                            start=(k == 0),
                            stop=(k == KC - 1),
                        )
                # half-wide extract: ACT has ~530ns fixed cost per
                # instruction, so one full-width op per step beats a
                # chunk-local pipeline by ~5us of pure overhead
                hsl = slice(base, base + NVH[h])
                if h == 1:
                    # half B's raw projection moves to SBUF now so every
                    # phase-D PSUM pool can close before the sim pools open;
                    # its normalize tail is emitted in the QS phase, woven
                    # between the first A reduces
                    nc.scalar.copy(DrawB[:], psd[:])
                sq = sqD_pool.tile([128, NVH[0]], bf16, name="sqd", tag="sq")
                nc.scalar.activation(sq[:, 0:NVH[h]], psd[:], AF.Square)
                for (off, ln) in h_chunks:
                    ssd = ssD.tile([1, 512], f32, name="ssd", tag="ssd")
                    nc.tensor.matmul(
                        ssd[:, :ln], onescol[:], sq[:, off:off + ln],
                        start=True, stop=True,
                    )
                    nc.scalar.copy(ssqD_row[:, base + off:base + off + ln],
                                   ssd[:, :ln])
                nc.scalar.activation(rowtmp[0:1, hsl], ssqD_row[0:1, hsl],
                                     AF.Sqrt)
                if h == 1:
                    continue
                # ~51-ULP reciprocal (fp32-only op) + cast to bf16 for the
                # K=1 broadcast matmul (plenty next to bf16 sim rounding)
                nc.vector.reciprocal_approx_fast(
                    invnD32[0:1, hsl], rowtmp[0:1, hsl]
                )
                nc.scalar.copy(invnD_row[0:1, hsl], invnD32[0:1, hsl])
                bc = dt_pool.tile([128, NVH[0]], bf16, name="bcast_sb",
                                  tag="bc")
                for (off, ln) in h_chunks:
                    psb = psB.tile([128, 512], f32, name="psb", tag="psb")
                    nc.tensor.matmul(
                        psb[:, :ln], onesrow[:],
                        invnD_row[:, base + off:base + off + ln],
                        start=True, stop=True,
                    )
                    nc.scalar.copy(bc[:, off:off + ln], psb[:, :ln])
                nc.vector.tensor_tensor(
                    DTn[:, hsl], psd[:], bc[:, 0:NVH[h]], op=ALU.mult
                )

        # ---------- phase Q+S: project query chunks, sim tiles interleaved --
        # Q-projection chunk j covers sim tiles 4j..4j+3. Chunks are burst
        # two ahead of their tiles; the 512-col projection PSUM has its own
        # single bank (a burst holds it ~1 tile) so the two big sim tensors
        # ping-pong PE writes against DVE reduces without a third claimant.
        # PSUM budget: 2x3 (sim) + 1 (qproj) + 1 (ssq + psout windows) = 8.
        with (
            tc.tile_pool(name="psQS", bufs=2, space="PSUM") as psQS,
            tc.tile_pool(name="psQ", bufs=1, space="PSUM") as psQ,
            tc.tile_pool(name="psM", bufs=1, space="PSUM") as psM,
            tc.tile_pool(name="sqQ_pool", bufs=2) as sqQ_pool,
            tc.tile_pool(name="m_pool", bufs=4) as m_pool,
        ):
            # one bank holds the Q sumsq columns (cols 0:NTT) and two
            # rotating [BPT, DPC] psout windows (cols NTT:NTT+2*DPC)
            misc = psM.tile([128, 512], f32, name="misc")

            psq_live = {}
            mall_live = {}

            def project_mm(j):
                jg, r = divmod(j * 512, QG)
                psq = psQ.tile([128, 512], f32, name="psq", tag="psq")
                psq_live[j] = psq
                for k in range(KC):
                    nc.tensor.matmul(
                        psq[:, 0:512],
                        wt[:, k, :],
                        qts[(k, jg)][:, r:r + 512],
                        start=(k == 0),
                        stop=(k == KC - 1),
                    )

            def project(j, qt_on_vector=False):
                sl = slice(j * 512, (j + 1) * 512)
                psq = psq_live[j]
                if qt_on_vector:
                    # ramp only: the DVE is idle before the first reduce and
                    # this copy gates the first sim tile
                    nc.vector.tensor_scalar_mul(QT[:, sl], psq[:, 0:512], 1.0)
                else:
                    nc.scalar.copy(QT[:, sl], psq[:, 0:512])
                project_norm(j)

            def project_qt0(j):
                sl = slice(j * 512, (j + 1) * 512)
                nc.vector.tensor_scalar_mul(QT[:, sl], psq_live[j][:, 0:512],
                                            1.0)

            def project_norm(j):
                psq = psq_live.pop(j)
                sq = sqQ_pool.tile([128, 512], bf16, name="sqq", tag="sqq")
                nc.scalar.activation(sq[:], psq[:, 0:512], AF.Square)
                for sx in range(4):
                    col = j * 4 + sx
                    nc.tensor.matmul(
                        misc[:, col:col + 1],
                        sq[:, sx * 128:(sx + 1) * 128],
                        onescol[:],
                        start=True,
                        stop=True,
                    )
                # per-chunk 1/|Q| and the weighted block-ones lhsT
                csl = slice(j * 4, (j + 1) * 4)
                nc.scalar.activation(normQ[:, csl], misc[:, csl], AF.Sqrt)
                nc.vector.reciprocal(invnQ[:, csl], normQ[:, csl])
                nc.vector.tensor_tensor(
                    lhsQ[:, csl, :],
                    qso[:].unsqueeze(1).broadcast_to((128, 4, BPT)),
                    invnQ[:, csl].unsqueeze(2).broadcast_to((128, 4, BPT)),
                    op=ALU.mult,
                )

            def simhalf(t, h):
                lq = QT[:, t * 128:(t + 1) * 128]
                if h == 0:
                    mall_live[t] = m_pool.tile([128, DPC], f32, name="mall",
                                               tag="mall")
                mall = mall_live[t]
                base = HB[h]
                ps = psQS.tile([128, NVH[0]], f32, name="pssim", tag="big")
                for (off, ln) in _chunks(NVH[h], 512):
                    nc.tensor.matmul(
                        ps[:, off:off + ln],
                        lq,
                        DTn[:, base + off:base + off + ln],
                        start=True,
                        stop=True,
                    )
                nc.vector.reduce_max(
                    mall[:, h * (DPC // 2):(h + 1) * (DPC // 2)],
                    ps[:, 0:NVH[h]].rearrange("p (g v) -> p g v", v=NVS[h]),
                    axis=AX.X,
                )

            def b_finish_recip():
                hsl = slice(HB[1], HB[1] + NVH[1])
                nc.vector.reciprocal_approx_fast(
                    invnD32[0:1, hsl], rowtmp[0:1, hsl]
                )
                nc.scalar.copy(invnD_row[0:1, hsl], invnD32[0:1, hsl])

            def b_finish_scale():
                # the K=1 broadcast borrows the q-projection PSUM slot; the
                # scale multiply reads the SBUF copy of B's raw projection
                bcB = sqQ_pool.tile([128, NVH[1]], bf16, name="bcB", tag="bcB")
                for (off, ln) in _chunks(NVH[1], 512):
                    psb = psQ.tile([128, 512], f32, name="psq", tag="psq")
                    nc.tensor.matmul(
                        psb[:, :ln], onesrow[:],
                        invnD_row[:, HB[1] + off:HB[1] + off + ln],
                        start=True, stop=True,
                    )
                    nc.scalar.copy(bcB[:, off:off + ln], psb[:, :ln])
                nc.vector.tensor_tensor(
                    DTn[:, HB[1]:HB[1] + NVH[1]], DrawB[:], bcB[:],
                    op=ALU.mult,
                )

            def finish_tile(t):
                mall = mall_live.pop(t)
                w = NTT + (t % 2) * DPC
                nc.tensor.matmul(
                    misc[0:BPT, w:w + DPC],
                    lhsQ[:, t, :],
                    mall[:],
                    start=True,
                    stop=True,
                )
                nc.scalar.copy(
                    outstage[:, t * DPC:(t + 1) * DPC], misc[0:BPT, w:w + DPC]
                )

            # tiles 0-3 ride chunk 0; ALL FOUR A-halves run before any
            # B-half — half B's DTn lands ~5us after half A's, and four A
            # reduces (~5.6us) cover exactly that window. Q-projection runs
            # one chunk ahead (its ~2.5us chain vs a ~10.5us chunk of sims).
            project_mm(0)
            project_qt0(0)
            simhalf(0, 0)
            simhalf(1, 0)
            b_finish_recip()
            project_norm(0)
            simhalf(2, 0)
            simhalf(3, 0)
            b_finish_scale()
            project_mm(1)
            simhalf(0, 1)
            simhalf(1, 1)
            project(1)
            simhalf(2, 1)
            simhalf(3, 1)
            finish_tile(0)
            finish_tile(1)
            finish_tile(2)
            finish_tile(3)
            for j in range(1, NQCH):
                if 2 * (j // 2) + 2 < NQCH and j % 2 == 0:
                    load_jg(j // 2 + 1)
                for ti, t in enumerate(range(j * 4, (j + 1) * 4)):
                    simhalf(t, 0)
                    if ti == 1 and j + 1 < NQCH:
                        project_mm(j + 1)
                    simhalf(t, 1)
                    finish_tile(t)
                    if ti == 2 and j + 1 < NQCH:
                        project(j + 1)
            nc.sync.dma_start(
                out_d[:, :].rearrange("(t f) c -> f t c", f=BPT),
                outstage[:].rearrange("f (t c) -> f t c", c=DPC),
            )
        qs_stack.__exit__(None, None, None)

    nc.compile()
    return nc


def _host_prep(q_hidden, d_hidden, W, d_mask):
    import ml_dtypes

    q = np.ascontiguousarray(np.asarray(q_hidden, dtype=np.float32))
    d = np.ascontiguousarray(np.asarray(d_hidden, dtype=np.float32))
    w = np.ascontiguousarray(np.asarray(W, dtype=np.float32))
    mask = np.asarray(d_mask, dtype=bool)

    def _pad8(x):
        x = max(int(x), 16)
        return min(int(-(-x // 8) * 8), ((LD + 7) // 8) * 8)

    # two length classes: sort docs by valid-token count, the 64 longest go
    # to each core's half A (padded to the global max), the 64 shortest to
    # half B (padded to the 65th-longest count) — ~8% fewer sim/reduce
    # elements than uniform padding, same instruction count
    nv = mask.sum(axis=1)
    order = np.argsort(-nv, kind="stable")
    NA = B // 2
    NV_A = _pad8(nv[order[0]])
    NV_B = _pad8(nv[order[NA]])
    # core m scores docs docids[m] (8 A-class then 8 B-class), in order
    docids = [
        np.concatenate([order[m * 8:(m + 1) * 8],
                        order[NA + m * 8:NA + (m + 1) * 8]])
        for m in range(NCORES)
    ]

    def _gather(c, NV):
        # valid tokens first, padded with dups of the first valid token
        # (duplicates never change a max)
        v = np.flatnonzero(mask[c])
        row = np.full(NV, v[0], dtype=np.intp)
        row[:min(len(v), NV)] = v[:NV]
        return d[c, row, :]                         # [NV, HID]

    bf = ml_dtypes.bfloat16
    qT = np.ascontiguousarray(q.reshape(TQ, HID).T.astype(bf))   # [HID, TQ]
    # W.T rearranged so the [128, KC, DIM] SBUF tile is one contiguous DMA:
    # wTp[p, k, d] = W[d, k*128+p]
    wT = np.ascontiguousarray(
        w.T.reshape(KC, 128, DIM).transpose(1, 0, 2).astype(bf)
    )
    dT_cores = []
    for m in range(NCORES):
        blk = np.concatenate(
            [_gather(c, NV_A) for c in docids[m][:8]]
            + [_gather(c, NV_B) for c in docids[m][8:]]
        )                                           # [8*NV_A + 8*NV_B, HID]
        dT_cores.append(np.ascontiguousarray(blk.T.astype(bf)))

    qso = np.zeros((128, 128 // LQ), dtype=np.float32)
    for p in range(128):
        qso[p, p // LQ] = 1.0
    onescol = np.ones((128, 1), dtype=bf)
    onesrow = np.ones((1, 128), dtype=bf)
    return NV_A, NV_B, docids, qT, wT, dT_cores, qso, onescol, onesrow


def kernel(q_hidden, d_hidden, W, d_mask):
    from concourse.bass_utils import run_bass_kernel_spmd

    NV_A, NV_B, docids, qT, wT, dT_cores, qso, onescol, onesrow = _host_prep(
        q_hidden, d_hidden, W, d_mask
    )
    nc = _build_program(NV_A, NV_B)

    in_maps = [
        {
            "qT": qT,
            "dT": dT_cores[m],
            "wT": wT,
            "qso": qso,
            "onescol": onescol,
            "onesrow": onesrow,
        }
        for m in range(NCORES)
    ]
    res = run_bass_kernel_spmd(nc, in_maps, core_ids=list(range(NCORES)))
    out = np.empty((B, B), dtype=np.float32)
    for m in range(NCORES):
        out[:, docids[m]] = res.results[m]["out"]
    return np.ascontiguousarray(out)


# revision 22
# speedup vs baseline: 1.0068x; 1.0068x over previous
"""ColBERT intra-batch MaxSim scoring kernel for 8 Trainium2 NeuronCores.

Math (see reference):
  Q = l2norm(q_hidden @ W.T)                       [B, LQ, DIM]
  D = l2norm(d_hidden @ W.T); D masked             [B, LD, DIM]
  sim[b,c,q,k] = Q[b,q]·D[c,k]; masked k -> -inf
  out[b,c] = sum_q max_k sim

Sharding: docs (dim c) are sharded 16-per-core; q_hidden/W replicated.
Each core computes its [B, 16] slice of the score matrix.

Design notes (v2):
  * The DVE reduce_max over the sim matrix is the hard floor: every sim
    element must pass through the vector engine once at ~1 elem/cycle/lane
    (~81us/core for 4096 q-tokens x 2432 doc-token cols / 128 lanes); no
    other engine can reduce along the free axis from PSUM. The kernel is
    therefore organized so DVE runs reduces back-to-back and everything
    else hides under them.
  * ALL activations/weights move and multiply in bf16 (fp32 PE matmuls are
    ~3.5x slower and fp32 DMA is 2x the bytes); PSUM accumulation stays
    fp32. Host casts inputs to bf16 after transposing.
  * Host pre-transposes activations to [HID, tokens] so every matmul has
    its contraction dim on partitions. The doc mask is folded away on the
    host: valid tokens gathered front, tail padded with dup of the first
    valid token (dups never change a max) -> no masking on device.
  * Q is NOT normalized before the sim matmul: max_k is invariant under a
    positive per-query scale, so 1/|Q| is folded into the block-ones
    lhsT of the final query-sum matmul.
  * D IS normalized before the sim matmul (1/|d_k| does not commute with
    max_k): ones-matmul sumsq -> sqrt (ACT) + fast reciprocal (DVE) ->
    K=1 ones outer-product matmul broadcasts 1/|D| to 128 partitions ->
    DVE multiply straight out of the projection PSUM into bf16 SBUF.
  * DVE reads at most ONE PSUM operand per instruction (HW rule), and
    only ~1 elem/cycle in every mode, so no fold tricks help; the plain
    grouped reduce (one per sim half-tile) is optimal.
"""

import os

import numpy as np

B, LQ, LD, HID, DIM = 128, 32, 256, 768, 128
NCORES = 8
DPC = B // NCORES          # docs per core
TQ = B * LQ                # total query tokens
KC = HID // 128            # contraction chunks for the projection


def _chunks(total, step):
    """[(off, len)] cut at `step` boundaries — a matmul's PSUM output must
    stay inside a single 512-float bank, so chunks may never straddle one."""
    return [(o, min(step, total - o)) for o in range(0, total, step)]


def _build_program(NV_A, NV_B):
    import concourse.bass as bass  # noqa: F401
    import concourse.tile as tile
    from concourse import bacc, mybir

    f32 = mybir.dt.float32
    bf16 = mybir.dt.bfloat16
    AF = mybir.ActivationFunctionType
    AX = mybir.AxisListType
    ALU = mybir.AluOpType

    # two doc classes: the 8 longest docs (padded to NV_A) in half A, the 8
    # shortest (padded to NV_B <= NV_A) in half B — same instruction count,
    # ~8% fewer sim/reduce elements than uniform padding
    NVH = [(DPC // 2) * NV_A, (DPC // 2) * NV_B]
    HB = [0, NVH[0]]               # half base offsets
    NVS = [NV_A, NV_B]
    NVT = NVH[0] + NVH[1]          # compacted doc tokens per core
    NQCH = TQ // 512        # q-projection column chunks
    NTT = TQ // 128         # sim lhsT tiles (query-token tiles)
    BPT = 128 // LQ         # batch entries per query-token tile
    QG = 1024               # qt DMA column-group width

    nc = bacc.Bacc(
        "TRN2",
        target_bir_lowering=False,
        debug=False,
        num_devices=NCORES,
    )

    qT_d = nc.dram_tensor("qT", [HID, TQ], bf16, kind="ExternalInput")
    dT_d = nc.dram_tensor("dT", [HID, NVT], bf16, kind="ExternalInput")
    wT_d = nc.dram_tensor("wT", [128, KC, DIM], bf16, kind="ExternalInput")
    qso_d = nc.dram_tensor("qso", [128, BPT], f32, kind="ExternalInput")
    onescol_d = nc.dram_tensor("onescol", [128, 1], bf16, kind="ExternalInput")
    onesrow_d = nc.dram_tensor("onesrow", [1, 128], bf16, kind="ExternalInput")
    out_d = nc.dram_tensor("out", [B, DPC], f32, kind="ExternalOutput")

    with tile.TileContext(nc) as tc, tc.tile_pool(name="persist", bufs=1) as per:
        # --- constants + persistent SBUF tensors ---------------------------
        wt = per.tile([128, KC, DIM], bf16, name="wt")
        qso = per.tile([128, BPT], f32, name="qso")
        onescol = per.tile([128, 1], bf16, name="onescol")
        onesrow = per.tile([1, 128], bf16, name="onesrow")
        QT = per.tile([128, TQ], bf16, name="QT")         # q-proj [d, t] unnormalized
        DTn = per.tile([128, NVT], bf16, name="DTn")      # normalized d-proj
        invnQ = per.tile([128, NTT], f32, name="invnQ")   # 1/|Q| per query token
        normQ = per.tile([128, NTT], f32, name="normQ")
        lhsQ = per.tile([128, NTT, BPT], f32, name="lhsQ")  # blockones * 1/|Q|
        ssqD_row = per.tile([1, NVT], f32, name="ssqD_row")
        invnD32 = per.tile([1, NVT], f32, name="invnD32")
        invnD_row = per.tile([1, NVT], bf16, name="invnD_row")
        rowtmp = per.tile([1, NVT], f32, name="rowtmp")
        DrawB = per.tile([128, NVH[1]], bf16, name="DrawB")  # raw B proj
        outstage = per.tile([BPT, NTT * DPC], f32, name="outstage")

        # constants go first on the gpsimd queue so wt is resident before
        # the first projection matmul; dT halves ride two queues in parallel
        # (sync: half A, gpsimd: half B) so phase D is DMA-gated for only
        # ~2MB per queue; qt jg0 rides the otherwise-idle scalar queue
        nc.gpsimd.dma_start(wt[:], wT_d[:, :, :])
        nc.gpsimd.dma_start(qso[:], qso_d[:, :])
        nc.gpsimd.dma_start(onescol[:], onescol_d[:, :])
        nc.gpsimd.dma_start(onesrow[:], onesrow_d[:, :])

        # ---------------- phase D: project + normalize doc tokens ---------
        # dT halves land on the sync queue (nothing else competes there);
        # qt column groups land on the gpsimd queue. Scalar/vector issue no
        # DMAs — their cycles belong to copies and reduces.
        qs_stack = tc.tile_pool(name="qt_pool", bufs=1)
        qt_pool = qs_stack.__enter__()
        qts = {}

        def load_jg(jg, engs=None):
            engs = engs or [nc.gpsimd]
            for k in range(KC):
                t_ = qt_pool.tile(
                    [128, QG], bf16, name=f"qt{k}_{jg}", tag=f"qt{k}",
                    bufs=2,
                )
                engs[k % len(engs)].dma_start(
                    t_[:], qT_d[k * 128:(k + 1) * 128, jg * QG:(jg + 1) * QG]
                )
                qts[(k, jg)] = t_

        # NOTE: scalar may only issue a FEW upfront DMAs — a back-pressured
        # issue blocks its in-order queue and stalls the phase-D squares
        # (measured +25us on the ramp with 12 queued issues).

        psQ_stack = tc.tile_pool(name="psQ", bufs=1, space="PSUM")
        psQ = psQ_stack.__enter__()
        with (
            tc.tile_pool(name="dt_pool", bufs=1) as dt_pool,
            tc.tile_pool(name="psD", bufs=1, space="PSUM") as psD,
            tc.tile_pool(name="ssD", bufs=1, space="PSUM") as ssD,
            tc.tile_pool(name="sqD_pool", bufs=2) as sqD_pool,
            tc.tile_pool(name="psB", bufs=1, space="PSUM") as psB,
        ):
            # Per-queue DMA rate is only ~60-90GB/s (small strided rows),
            # so only ramp-critical bytes go upfront, spread over all three
            # queues: dT half A + qt group 0, then dT half B + qt group 1;
            # later qt groups trickle mid-loop on gpsimd. Scalar's issue
            # count stays small so its in-order ACT queue isn't blocked.
            dts = {}
            for h in range(2):
                for k in range(KC):
                    dts[(k, h)] = dt_pool.tile(
                        [128, NVH[h]], bf16, name=f"dt{k}_{h}", tag=f"dt{k}_{h}"
                    )

            def load_dt(h, ks, eng):
                for k in ks:
                    eng.dma_start(
                        dts[(k, h)][:],
                        dT_d[k * 128:(k + 1) * 128, HB[h]:HB[h] + NVH[h]],
                    )

            def load_jg_ks(jg, ks, eng):
                for k in ks:
                    t_ = qt_pool.tile(
                        [128, QG], bf16, name=f"qt{k}_{jg}", tag=f"qt{k}",
                        bufs=2,
                    )
                    eng.dma_start(
                        t_[:], qT_d[k * 128:(k + 1) * 128,
                                    jg * QG:(jg + 1) * QG]
                    )
                    qts[(k, jg)] = t_

            load_dt(0, [4, 5], nc.scalar)
            load_dt(0, [0, 2], nc.sync)
            load_dt(0, [1, 3], nc.gpsimd)
            load_jg_ks(0, [0, 1], nc.scalar)
            load_jg_ks(0, [2, 3], nc.sync)
            load_jg_ks(0, [4, 5], nc.gpsimd)
            load_dt(1, [0, 2, 4], nc.sync)
            load_dt(1, [1, 3, 5], nc.gpsimd)
            load_jg_ks(1, [0, 1, 2], nc.scalar)
            load_jg_ks(1, [3, 4, 5], nc.sync)

            psq_live = {}

            def project_mm(j):
                jg, r = divmod(j * 512, QG)
                psq = psQ.tile([128, 512], f32, name="psq", tag="psq")
                psq_live[j] = psq
                for k in range(KC):
                    nc.tensor.matmul(
                        psq[:, 0:512],
                        wt[:, k, :],
                        qts[(k, jg)][:, r:r + 512],
                        start=(k == 0),
                        stop=(k == KC - 1),
                    )

            for h in range(2):
                if h == 1:
                    # chunk-0 q-projection + the DVE copy of its QT columns
                    # ride ahead of half B, then a scheduler-only fence keeps
                    # B's data-gated matmuls from head-of-line-blocking the
                    # half-A extract chain on the PE
                    project_mm(0)
                    nc.vector.tensor_scalar_mul(QT[:, 0:512],
                                                psq_live[0][:, 0:512], 1.0)
                    tc.no_sync_barrier()
                base = HB[h]
                h_chunks = _chunks(NVH[h], 512)
                psd = psD.tile([128, NVH[h]], f32, name="psd", tag=f"psd{h}")
                for k in range(KC):
                    for (off, ln) in h_chunks:
                        nc.tensor.matmul(
                            psd[:, off:off + ln],
                            wt[:, k, :],
                            dts[(k, h)][:, off:off + ln],
                            start=(k == 0),
                            stop=(k == KC - 1),
                        )
                # half-wide extract: ACT has ~530ns fixed cost per
                # instruction, so one full-width op per step beats a
                # chunk-local pipeline by ~5us of pure overhead
                hsl = slice(base, base + NVH[h])
                if h == 1:
                    # half B's raw projection moves to SBUF now so every
                    # phase-D PSUM pool can close before the sim pools open;
                    # its normalize tail is emitted in the QS phase, woven
                    # between the first A reduces
                    nc.scalar.copy(DrawB[:], psd[:])
                sq = sqD_pool.tile([128, NVH[0]], bf16, name="sqd", tag="sq")
                nc.scalar.activation(sq[:, 0:NVH[h]], psd[:], AF.Square)
                for (off, ln) in h_chunks:
                    ssd = ssD.tile([1, 512], f32, name="ssd", tag="ssd")
                    nc.tensor.matmul(
                        ssd[:, :ln], onescol[:], sq[:, off:off + ln],
                        start=True, stop=True,
                    )
                    nc.scalar.copy(ssqD_row[:, base + off:base + off + ln],
                                   ssd[:, :ln])
                nc.scalar.activation(rowtmp[0:1, hsl], ssqD_row[0:1, hsl],
                                     AF.Sqrt)
                if h == 1:
                    continue
                # ~51-ULP reciprocal (fp32-only op) + cast to bf16 for the
                # K=1 broadcast matmul (plenty next to bf16 sim rounding)
                nc.vector.reciprocal_approx_fast(
                    invnD32[0:1, hsl], rowtmp[0:1, hsl]
                )
                nc.scalar.copy(invnD_row[0:1, hsl], invnD32[0:1, hsl])
                bc = dt_pool.tile([128, NVH[0]], bf16, name="bcast_sb",
                                  tag="bc")
                for (off, ln) in h_chunks:
                    psb = psB.tile([128, 512], f32, name="psb", tag="psb")
                    nc.tensor.matmul(
                        psb[:, :ln], onesrow[:],
                        invnD_row[:, base + off:base + off + ln],
                        start=True, stop=True,
                    )
                    nc.scalar.copy(bc[:, off:off + ln], psb[:, :ln])
                nc.vector.tensor_tensor(
                    DTn[:, hsl], psd[:], bc[:, 0:NVH[h]], op=ALU.mult
                )

        # ---------- phase Q+S: project query chunks, sim tiles interleaved --
        # Q-projection chunk j covers sim tiles 4j..4j+3. Chunks are burst
        # two ahead of their tiles; the 512-col projection PSUM has its own
        # single bank (a burst holds it ~1 tile) so the two big sim tensors
        # ping-pong PE writes against DVE reduces without a third claimant.
        # PSUM budget: 2x3 (sim) + 1 (qproj) + 1 (ssq + psout windows) = 8.
        with (
            tc.tile_pool(name="psQS", bufs=2, space="PSUM") as psQS,
            tc.tile_pool(name="psM", bufs=1, space="PSUM") as psM,
            tc.tile_pool(name="sqQ_pool", bufs=2) as sqQ_pool,
            tc.tile_pool(name="m_pool", bufs=4) as m_pool,
        ):
            # one bank holds the Q sumsq columns (cols 0:NTT) and two
            # rotating [BPT, DPC] psout windows (cols NTT:NTT+2*DPC)
            misc = psM.tile([128, 512], f32, name="misc")

            mall_live = {}

            def project(j, qt_on_vector=False):
                sl = slice(j * 512, (j + 1) * 512)
                psq = psq_live[j]
                if qt_on_vector:
                    # ramp only: the DVE is idle before the first reduce and
                    # this copy gates the first sim tile
                    nc.vector.tensor_scalar_mul(QT[:, sl], psq[:, 0:512], 1.0)
                else:
                    nc.scalar.copy(QT[:, sl], psq[:, 0:512])
                project_norm(j)

            def project_norm(j):
                psq = psq_live.pop(j)
                sq = sqQ_pool.tile([128, 512], bf16, name="sqq", tag="sqq")
                nc.scalar.activation(sq[:], psq[:, 0:512], AF.Square)
                for sx in range(4):
                    col = j * 4 + sx
                    nc.tensor.matmul(
                        misc[:, col:col + 1],
                        sq[:, sx * 128:(sx + 1) * 128],
                        onescol[:],
                        start=True,
                        stop=True,
                    )
                # per-chunk 1/|Q| and the weighted block-ones lhsT
                csl = slice(j * 4, (j + 1) * 4)
                nc.scalar.activation(normQ[:, csl], misc[:, csl], AF.Sqrt)
                nc.vector.reciprocal(invnQ[:, csl], normQ[:, csl])
                nc.vector.tensor_tensor(
                    lhsQ[:, csl, :],
                    qso[:].unsqueeze(1).broadcast_to((128, 4, BPT)),
                    invnQ[:, csl].unsqueeze(2).broadcast_to((128, 4, BPT)),
                    op=ALU.mult,
                )

            def simhalf(t, h):
                lq = QT[:, t * 128:(t + 1) * 128]
                if h == 0:
                    mall_live[t] = m_pool.tile([128, DPC], f32, name="mall",
                                               tag="mall")
                mall = mall_live[t]
                base = HB[h]
                ps = psQS.tile([128, NVH[0]], f32, name="pssim", tag="big")
                for (off, ln) in _chunks(NVH[h], 512):
                    nc.tensor.matmul(
                        ps[:, off:off + ln],
                        lq,
                        DTn[:, base + off:base + off + ln],
                        start=True,
                        stop=True,
                    )
                nc.vector.reduce_max(
                    mall[:, h * (DPC // 2):(h + 1) * (DPC // 2)],
                    ps[:, 0:NVH[h]].rearrange("p (g v) -> p g v", v=NVS[h]),
                    axis=AX.X,
                )

            def b_finish_recip():
                hsl = slice(HB[1], HB[1] + NVH[1])
                nc.vector.reciprocal_approx_fast(
                    invnD32[0:1, hsl], rowtmp[0:1, hsl]
                )
                nc.scalar.copy(invnD_row[0:1, hsl], invnD32[0:1, hsl])

            def b_finish_scale():
                # the K=1 broadcast borrows the q-projection PSUM slot; the
                # scale multiply reads the SBUF copy of B's raw projection
                bcB = sqQ_pool.tile([128, NVH[1]], bf16, name="bcB", tag="bcB")
                for (off, ln) in _chunks(NVH[1], 512):
                    psb = psQ.tile([128, 512], f32, name="psq", tag="psq")
                    nc.tensor.matmul(
                        psb[:, :ln], onesrow[:],
                        invnD_row[:, HB[1] + off:HB[1] + off + ln],
                        start=True, stop=True,
                    )
                    nc.scalar.copy(bcB[:, off:off + ln], psb[:, :ln])
                nc.vector.tensor_tensor(
                    DTn[:, HB[1]:HB[1] + NVH[1]], DrawB[:], bcB[:],
                    op=ALU.mult,
                )

            def finish_tile(t):
                mall = mall_live.pop(t)
                w = NTT + (t % 2) * DPC
                nc.tensor.matmul(
                    misc[0:BPT, w:w + DPC],
                    lhsQ[:, t, :],
                    mall[:],
                    start=True,
                    stop=True,
                )
                nc.scalar.copy(
                    outstage[:, t * DPC:(t + 1) * DPC], misc[0:BPT, w:w + DPC]
                )

            # tiles 0-3 ride chunk 0; ALL FOUR A-halves run before any
            # B-half — half B's DTn lands ~5us after half A's, and four A
            # reduces (~5.6us) cover exactly that window. Q-projection runs
            # one chunk ahead (its ~2.5us chain vs a ~10.5us chunk of sims).
            simhalf(0, 0)
            simhalf(1, 0)
            b_finish_recip()
            project_norm(0)
            simhalf(2, 0)
            simhalf(3, 0)
            b_finish_scale()
            project_mm(1)
            load_jg_ks(2, [0, 1, 2, 3, 4, 5], nc.gpsimd)
            load_jg_ks(3, [0, 1, 2, 3, 4, 5], nc.gpsimd)
            simhalf(0, 1)
            simhalf(1, 1)
            project(1)
            simhalf(2, 1)
            simhalf(3, 1)
            finish_tile(0)
            finish_tile(1)
            finish_tile(2)
            finish_tile(3)
            for j in range(1, NQCH):
                if 2 * (j // 2) + 2 < NQCH and j % 2 == 0:
                    load_jg(j // 2 + 1)
                for ti, t in enumerate(range(j * 4, (j + 1) * 4)):
                    simhalf(t, 0)
                    if ti == 1 and j + 1 < NQCH:
                        project_mm(j + 1)
                    simhalf(t, 1)
                    finish_tile(t)
                    if ti == 2 and j + 1 < NQCH:
                        project(j + 1)
            nc.sync.dma_start(
                out_d[:, :].rearrange("(t f) c -> f t c", f=BPT),
                outstage[:].rearrange("f (t c) -> f t c", c=DPC),
            )
        psQ_stack.__exit__(None, None, None)
        qs_stack.__exit__(None, None, None)

    nc.compile()
    return nc


def _host_prep(q_hidden, d_hidden, W, d_mask):
    import ml_dtypes

    q = np.ascontiguousarray(np.asarray(q_hidden, dtype=np.float32))
    d = np.ascontiguousarray(np.asarray(d_hidden, dtype=np.float32))
    w = np.ascontiguousarray(np.asarray(W, dtype=np.float32))
    mask = np.asarray(d_mask, dtype=bool)

    def _pad8(x):
        x = max(int(x), 16)
        return min(int(-(-x // 8) * 8), ((LD + 7) // 8) * 8)

    # two length classes: sort docs by valid-token count, the 64 longest go
    # to each core's half A (padded to the global max), the 64 shortest to
    # half B (padded to the 65th-longest count) — ~8% fewer sim/reduce
    # elements than uniform padding, same instruction count
    nv = mask.sum(axis=1)
    order = np.argsort(-nv, kind="stable")
    NA = B // 2
    NV_A = _pad8(nv[order[0]])
    NV_B = _pad8(nv[order[NA]])
    # core m scores docs docids[m] (8 A-class then 8 B-class), in order
    docids = [
        np.concatenate([order[m * 8:(m + 1) * 8],
                        order[NA + m * 8:NA + (m + 1) * 8]])
        for m in range(NCORES)
    ]

    def _gather(c, NV):
        # valid tokens first, padded with dups of the first valid token
        # (duplicates never change a max)
        v = np.flatnonzero(mask[c])
        row = np.full(NV, v[0], dtype=np.intp)
        row[:min(len(v), NV)] = v[:NV]
        return d[c, row, :]                         # [NV, HID]

    bf = ml_dtypes.bfloat16
    qT = np.ascontiguousarray(q.reshape(TQ, HID).T.astype(bf))   # [HID, TQ]
    # W.T rearranged so the [128, KC, DIM] SBUF tile is one contiguous DMA:
    # wTp[p, k, d] = W[d, k*128+p]
    wT = np.ascontiguousarray(
        w.T.reshape(KC, 128, DIM).transpose(1, 0, 2).astype(bf)
    )
    dT_cores = []
    for m in range(NCORES):
        blk = np.concatenate(
            [_gather(c, NV_A) for c in docids[m][:8]]
            + [_gather(c, NV_B) for c in docids[m][8:]]
        )                                           # [8*NV_A + 8*NV_B, HID]
        dT_cores.append(np.ascontiguousarray(blk.T.astype(bf)))

    qso = np.zeros((128, 128 // LQ), dtype=np.float32)
    for p in range(128):
        qso[p, p // LQ] = 1.0
    onescol = np.ones((128, 1), dtype=bf)
    onesrow = np.ones((1, 128), dtype=bf)
    return NV_A, NV_B, docids, qT, wT, dT_cores, qso, onescol, onesrow


def kernel(q_hidden, d_hidden, W, d_mask):
    from concourse.bass_utils import run_bass_kernel_spmd

    NV_A, NV_B, docids, qT, wT, dT_cores, qso, onescol, onesrow = _host_prep(
        q_hidden, d_hidden, W, d_mask
    )
    nc = _build_program(NV_A, NV_B)

    in_maps = [
        {
            "qT": qT,
            "dT": dT_cores[m],
            "wT": wT,
            "qso": qso,
            "onescol": onescol,
            "onesrow": onesrow,
        }
        for m in range(NCORES)
    ]
    res = run_bass_kernel_spmd(nc, in_maps, core_ids=list(range(NCORES)))
    out = np.empty((B, B), dtype=np.float32)
    for m in range(NCORES):
        out[:, docids[m]] = res.results[m]["out"]
    return np.ascontiguousarray(out)
